# revision 5
# baseline (speedup 1.0000x reference)
"""Trainium2 Bass kernel for ColaViT pre-attention QKV down-projection.

Computes gelu(hidden_states @ concat(w_q, w_k, w_v)) and splits into
(q_low, k_low, v_low), matching the fp32 jax reference.

Sharding: data-parallel on batch across 8 NeuronCores; each core owns
M=1576 token rows of the [12608, 768] x [768, 576] GEMM + exact Gelu.

v4 strategy (from v2/v3 trace analysis):
- All inputs host-packed contiguous fp16; loads on sync HWDGE ring,
  stores on scalar HWDGE ring.
- w is loaded as SIX per-k slices and the first x chunk is a single
  m-tile, ordered x0, wk0-2, ... so the first real matmul can issue
  ~1.5us earlier than with w halves.
- PE warm-up (6 zero matmuls) runs back-to-back into the real stream
  with no idle gap: any PE-idle gap resets the free-running 3.4us HAM
  window and costs ~3-5us of 1.2 GHz streaming (v3's mistake).
- Per m-tile: 2-bank PSUM tile, 12 matmuls (k-major, n0/n1 pairs),
  one wide exact-Gelu ACTIVATE evicting both banks to fp16 SBUF.
- 40-row tail chunk processed second (its slow 40-partition store
  descriptor hides mid-stream); last chunk is one fast 128-row tile.
- Tail warm-up matmuls after the last real matmul keep the PE/NX warm
  through the final barrier into the NRT postamble (51 sem resets per
  engine) which otherwise runs at re-throttled dispatch rate.
- fp16 outputs; host un-permutes and casts to fp32.
"""

import numpy as np

HIDDEN = 768
RANK = 192
N_OUT = 3 * RANK          # 576
B, S = 64, 197
N_CORES = 8
M_PER_CORE = B * S // N_CORES   # 1576
P = 128
K_TILES = HIDDEN // P     # 6
N_CHUNK = 288             # one n-half
N_WARMUP_MM = 6
N_TAILWARM_MM = 8

# chunks in PROCESSING order: (row_offset, rows). 40-row tail second.
CHUNKS = [(0, P), (1536, 40), (128, 2 * P), (384, 3 * P),
          (768, 4 * P), (1280, P), (1408, P)]
assert sum(c[1] for c in CHUNKS) == M_PER_CORE

_CACHE = {}


def _build_nc():
    from contextlib import ExitStack

    import concourse.bacc as bacc
    import concourse.mybir as mybir
    from concourse.tile import TileContext

    f32 = mybir.dt.float32
    f16 = mybir.dt.float16
    gelu = mybir.ActivationFunctionType.Gelu

    nc = bacc.Bacc("TRN2", target_bir_lowering=False, debug=False,
                   num_devices=N_CORES)

    w_dram = [nc.dram_tensor(f"w{k}", [P, N_OUT], f16,
                             kind="ExternalInput") for k in range(K_TILES)]
    x_dram = [nc.dram_tensor(f"x{ci}", [P, K_TILES * csz], f16,
                             kind="ExternalInput")
              for ci, (_, csz) in enumerate(CHUNKS)]
    y_dram = []
    for ci, (_, csz) in enumerate(CHUNKS):
        if csz % P == 0:
            y_dram.append(nc.dram_tensor(f"y{ci}", [P, (csz // P) * N_OUT],
                                         f16, kind="ExternalOutput"))
        else:
            y_dram.append(nc.dram_tensor(f"y{ci}", [csz, N_OUT], f16,
                                         kind="ExternalOutput"))

    with TileContext(nc) as tc, ExitStack() as ctx:
        sb = ctx.enter_context(tc.tile_pool(name="sb", bufs=1))
        pp = ctx.enter_context(tc.tile_pool(name="pp", bufs=3, space="PSUM"))

        # PE warm-up: zero tile memset on gpsimd (free early), then a
        # burst of matmuls bridging until first data lands.
        zt = sb.tile([P, 520], f16, tag="zt", name="zt")
        nc.gpsimd.memset(zt[:], 0.0)
        zps = pp.tile([8, 512], f32, tag="zps", name="zps", bufs=1)
        for _ in range(N_WARMUP_MM):
            nc.tensor.matmul(zps[:], zt[:, :8], zt[:, 8:520],
                             start=True, stop=True)

        # loads on the sync HWDGE ring: first x chunk, then w k-slices
        # interleaved so matmuls can start as soon as wk0 lands.
        wt = [sb.tile([P, N_OUT], f16, tag=f"w{k}", name=f"w{k}")
              for k in range(K_TILES)]
        xt = [sb.tile([P, K_TILES, csz], f16, tag=f"x{ci}", name=f"x{ci}")
              for ci, (_, csz) in enumerate(CHUNKS)]

        def load_x(ci):
            nc.sync.dma_start(xt[ci][:], x_dram[ci][:].rearrange(
                "p (a m) -> p a m", a=K_TILES))

        load_x(0)
        for k in range(3):
            nc.sync.dma_start(wt[k][:], w_dram[k][:])
        load_x(1)
        for k in range(3, K_TILES):
            nc.sync.dma_start(wt[k][:], w_dram[k][:])
        for ci in range(2, len(CHUNKS)):
            load_x(ci)

        for ci, (c0, csz) in enumerate(CHUNKS):
            n_mt = (csz + P - 1) // P
            ysb = sb.tile([P, n_mt, N_OUT], f16, tag=f"ysb{ci}",
                          name=f"ysb{ci}")
            for mj in range(n_mt):
                msz = min(P, csz - mj * P)
                ml = mj * P
                ps = pp.tile([P, 2, 512], f32, tag="ps",
                             name=f"ps{ci}_{mj}")
                for k in range(K_TILES):
                    for nj in range(2):
                        nc.tensor.matmul(
                            ps[:msz, nj, :N_CHUNK],
                            xt[ci][:, k, ml:ml + msz],
                            wt[k][:, nj * N_CHUNK:(nj + 1) * N_CHUNK],
                            start=(k == 0),
                            stop=(k == K_TILES - 1),
                        )
                nc.scalar.activation(ysb[:msz, mj, :],
                                     ps[:msz, :, :N_CHUNK], gelu)
            if csz % P == 0:
                nc.scalar.dma_start(
                    y_dram[ci][:].rearrange("p (a n) -> p a n", a=n_mt),
                    ysb[:, :, :])
            else:
                nc.scalar.dma_start(y_dram[ci][:, :], ysb[:csz, 0, :])

        # keep PE/NX busy into the final barrier -> warm NRT postamble
        for _ in range(N_TAILWARM_MM):
            nc.tensor.matmul(zps[:], zt[:, :8], zt[:, 8:520],
                             start=True, stop=True)

    nc.compile()
    return nc


def _get_nc():
    if "nc" not in _CACHE:
        _CACHE["nc"] = _build_nc()
    return _CACHE["nc"]


def _make_in_maps(hidden_states, w_q, w_k, w_v):
    x = np.asarray(hidden_states, dtype=np.float32).reshape(B * S, HIDDEN)
    xT16 = np.ascontiguousarray(x.T).astype(np.float16)     # [768, 12608]
    wcat = np.concatenate(
        [np.asarray(w_q, np.float32), np.asarray(w_k, np.float32),
         np.asarray(w_v, np.float32)], axis=1).astype(np.float16)

    in_maps = []
    for c in range(N_CORES):
        base = c * M_PER_CORE
        m = {f"w{k}": np.ascontiguousarray(wcat[k * P:(k + 1) * P, :])
             for k in range(K_TILES)}
        for ci, (c0, csz) in enumerate(CHUNKS):
            seg = xT16[:, base + c0:base + c0 + csz]        # [768, csz]
            seg = seg.reshape(K_TILES, P, csz).transpose(1, 0, 2)
            m[f"x{ci}"] = np.ascontiguousarray(
                seg.reshape(P, K_TILES * csz))
        in_maps.append(m)
    return in_maps


def _postprocess(results):
    y_full = np.empty((B * S, N_OUT), dtype=np.float32)
    for c in range(N_CORES):
        base = c * M_PER_CORE
        res = results[c]
        for ci, (c0, csz) in enumerate(CHUNKS):
            buf = res[f"y{ci}"]
            if csz % P == 0:
                n_mt = csz // P
                seg = buf.reshape(P, n_mt, N_OUT).transpose(1, 0, 2)
                y_full[base + c0:base + c0 + csz, :] = \
                    seg.reshape(csz, N_OUT)
            else:
                y_full[base + c0:base + c0 + csz, :] = buf
    y_full = y_full.reshape(B, S, N_OUT)
    q = np.ascontiguousarray(y_full[:, :, :RANK])
    k = np.ascontiguousarray(y_full[:, :, RANK:2 * RANK])
    v = np.ascontiguousarray(y_full[:, :, 2 * RANK:])
    return (q, k, v)


def kernel(hidden_states, w_q, w_k, w_v):
    from concourse.bass_utils import run_bass_kernel_spmd

    nc = _get_nc()
    in_maps = _make_in_maps(hidden_states, w_q, w_k, w_v)
    res = run_bass_kernel_spmd(nc, in_maps, list(range(N_CORES)))
    return _postprocess(res.results)
